# revision 5
# baseline (speedup 1.0000x reference)
"""Trainium2 kernel for nn_GroupedStackedAFDF.

Every op in the reference (block-diagonal complex matmul, FFT, IFFT, channel
permutation) is linear along the channel axis with fixed weights, so the whole
4-layer network collapses into a single complex matrix T with
    out = Re(T @ z) = Re(T) @ x          (x is real)
T is built on host from the tiny weights (exact, complex128); the device then
runs one dense [32768,1024] @ [1024,1024] real matmul, data-parallel over the
batch dim across 8 cores (4096 rows/core).

Device layout: everything is computed transposed (channels on partitions):
    outT[ch_out, b] = W.T @ xT   with  W = Re(T).T  ([ch_in, ch_out])
The PE-pitch floor (512 matmuls x ~216 ns) dominates; the kernel is built to
compress everything around it:
  - x is staged k-subtile-major ([128, 8, NB]) so each 512-batch chunk loads
    as four 256 KB pair-DMAs; the first real matmul only needs w0 + one pair.
  - all load DMAs are issued on one queue in the exact order the PE consumes
    them (w0, x0 pairs, w1..w7, x1.., prefetch depth 3), so the globally
    serialized DMA transfers never let a later tile jump ahead of an
    earlier-needed one.
  - PSUM: all 8 banks rotate through one pool; PE warm-up matmuls (which ramp
    the clock out of the low p-state while the first loads are in flight) use
    the same pool.
  - outputs are written bf16 (copy converts f32 PSUM -> bf16 SBUF), halving
    store traffic and shortening the post-matmul drain tail.
"""

import numpy as np
import ml_dtypes

import concourse.bass as bass
from concourse import bacc
import concourse.mybir as mybir
from concourse.tile import TileContext
from concourse.bass_utils import run_bass_kernel_spmd

N, D, L, G = 32768, 1024, 4, 32
DG = D // G
NCORES = 8
NB = N // NCORES          # 4096 batch rows per core
BCH = 512                 # batch chunk = psum free dim
NKT = D // 128            # 8 contraction tiles
NMT = D // 128            # 8 output-channel tiles
NCH = NB // BCH           # 8 batch chunks per core
NPAIR = NKT // 2          # k-subtile pairs per chunk (DMA granularity)

_BF16 = mybir.dt.bfloat16
_F32 = mybir.dt.float32

WARM_FULL = 3             # 512-wide warm-up matmuls
WARM_SMALL = 5            # 128-wide warm-up matmuls (fine-grained tail)
XBUFS = 3                 # x chunk prefetch depth


def _build_T(Aa, Ab, Da, Db, perms):
    """Compose the network into one complex [D, D] matrix acting on channel
    vectors: z_out = T @ z_in."""
    T = np.eye(D, dtype=np.complex128)
    for l in range(L):
        Wa = Aa[l].astype(np.float64) + 1j * Ab[l].astype(np.float64)
        Wd = Da[l].astype(np.float64) + 1j * Db[l].astype(np.float64)
        T = np.einsum("gok,gkc->goc", Wa, T.reshape(G, DG, D)).reshape(D, D)
        T = np.fft.fft(T, axis=0)
        T = np.einsum("gok,gkc->goc", Wd, T.reshape(G, DG, D)).reshape(D, D)
        T = np.fft.ifft(T, axis=0)
        T = T[np.asarray(perms[l]), :]
    return T


def _build_nc():
    nc = bacc.Bacc("TRN2", target_bir_lowering=False, enable_partition_id=False)
    xT8 = nc.declare_dram_parameter("xT8", [128, NKT, NB], _BF16, isOutput=False)
    W = nc.declare_dram_parameter("W", [D, D], _BF16, isOutput=False)
    outT = nc.declare_dram_parameter("outT", [D, NB], _BF16, isOutput=True)

    with TileContext(nc) as tc:
        with (
            tc.tile_pool(name="wpool", bufs=1) as wpool,
            tc.tile_pool(name="xpool", bufs=XBUFS) as xpool,
            tc.tile_pool(name="pspool", bufs=8, space="PSUM") as pspool,
            tc.tile_pool(name="opool", bufs=4) as opool,
        ):
            # PE warm-up on a zeroed tile: keeps the PE clock ramping out of
            # the low p-state while the first loads are in flight.
            warm_x = wpool.tile([128, BCH], _BF16, tag="warmx", name="warm_x")
            nc.vector.memset(warm_x[:], 0.0)
            for i in range(WARM_FULL + WARM_SMALL):
                warm_ps = pspool.tile([128, BCH], _F32, tag="ps", name=f"wps{i}")
                wid = BCH if i < WARM_FULL else 128
                nc.tensor.matmul(
                    warm_ps[:, 0:wid], warm_x[:, 0:128], warm_x[:, 0:wid],
                    start=True, stop=True,
                )

            # W is pre-arranged on host so row-block m holds all 8 [128,128]
            # lhsT blocks for output-channel tile m side by side:
            #   W[m*128+p, k*128+q] = Wmat[k*128+p, m*128+q]
            # w0 goes on the scalar queue so its transfer races the first x
            # pair; every other load is issued on sync in exactly the order
            # the PE consumes it (transfers are globally serialized, so issue
            # order == landing order).
            wt = [None] * NMT
            w_tile = wpool.tile([128, D], _BF16, tag="w0", name="w0")
            nc.scalar.dma_start(out=w_tile[:], in_=W[0:128, :])
            wt[0] = w_tile

            xt = [[None] * NPAIR for _ in range(NCH)]
            for p in range(NPAIR):
                x_tile = xpool.tile([128, 2, BCH], _BF16, tag=f"x{p}", name=f"x{p}_0")
                nc.sync.dma_start(out=x_tile[:], in_=xT8[:, 2 * p : 2 * p + 2, 0:BCH])
                xt[0][p] = x_tile
            for m in range(1, NMT):
                w_tile = wpool.tile([128, D], _BF16, tag=f"w{m}", name=f"w{m}")
                nc.sync.dma_start(out=w_tile[:], in_=W[m * 128 : (m + 1) * 128, :])
                wt[m] = w_tile
            for b in range(1, NCH):
                bsl = slice(b * BCH, (b + 1) * BCH)
                for p in range(NPAIR):
                    x_tile = xpool.tile(
                        [128, 2, BCH], _BF16, tag=f"x{p}", name=f"x{p}_{b}"
                    )
                    nc.sync.dma_start(
                        out=x_tile[:], in_=xT8[:, 2 * p : 2 * p + 2, bsl]
                    )
                    xt[b][p] = x_tile

            for b in range(NCH):
                bsl = slice(b * BCH, (b + 1) * BCH)
                for m in range(NMT):
                    ps = pspool.tile([128, BCH], _F32, tag="ps", name=f"ps{b}_{m}")
                    msl = slice(m * 128, (m + 1) * 128)
                    # The very last tile runs as two 256-wide chains so the
                    # final copy + store (the post-matmul drain tail) are
                    # half-size.
                    last = b == NCH - 1 and m == NMT - 1
                    halves = 2 if last else 1
                    hw_ = BCH // halves
                    for h in range(halves):
                        hsl = slice(h * hw_, (h + 1) * hw_)
                        for k in range(NKT):
                            nc.tensor.matmul(
                                ps[:, hsl],
                                wt[m][:, k * 128 : (k + 1) * 128],
                                xt[b][k // 2][:, k % 2, hsl],
                                start=(k == 0),
                                stop=(k == NKT - 1),
                            )
                        o_tile = opool.tile(
                            [128, hw_], _BF16, tag=f"oh{h}" if last else "o",
                            name=f"o{b}_{m}_{h}",
                        )
                        nc.vector.tensor_copy(o_tile[:], ps[:, hsl])
                        nc.scalar.dma_start(
                            out=outT[msl, b * BCH + h * hw_ : b * BCH + (h + 1) * hw_],
                            in_=o_tile[:],
                        )
    nc.finalize()
    return nc


_nc_cache = {}


def _get_nc():
    if "nc" not in _nc_cache:
        _nc_cache["nc"] = _build_nc()
    return _nc_cache["nc"]


def _prep_x(x):
    """[N, D] f32 -> [NCORES, 128, NKT, NB] bf16, k-subtile-major per core:
    arr[c, p, k, b] = x[c*NB + b, k*128 + p]."""
    xr = x.reshape(NCORES, NB, NKT, 128).transpose(0, 3, 2, 1)
    return np.ascontiguousarray(xr).astype(ml_dtypes.bfloat16)


def _prep_W(T):
    """bf16 weights, rearranged m-major: W[m*128+p, k*128+q] = Re(T).T[k*128+p, m*128+q]."""
    Wmat = np.real(T).T.astype(ml_dtypes.bfloat16)       # [ch_in, ch_out]
    return np.ascontiguousarray(
        Wmat.reshape(NKT, 128, NMT, 128).transpose(2, 1, 0, 3).reshape(D, D)
    )


def _run_device(x8, W_bf16, trace=False, **kw):
    """x8: [NCORES, 128, NKT, NB] bf16, W_bf16: [D, D] bf16.
    Returns (out [N, D] f32, result)."""
    nc = _get_nc()
    in_maps = [{"xT8": x8[c], "W": W_bf16} for c in range(NCORES)]
    try:
        res = run_bass_kernel_spmd(nc, in_maps, list(range(NCORES)), trace=trace, **kw)
    except Exception:
        # transient NRT/device hiccups have been observed; retry once
        res = run_bass_kernel_spmd(nc, in_maps, list(range(NCORES)), trace=trace, **kw)
    out = np.empty((N, D), np.float32)
    for c in range(NCORES):
        out[c * NB : (c + 1) * NB, :] = res.results[c]["outT"].T.astype(np.float32)
    return out, res


def kernel(x, Aa, Ab, Da, Db, perms):
    x = np.asarray(x, dtype=np.float32)
    Aa, Ab, Da, Db = (np.asarray(a, dtype=np.float32) for a in (Aa, Ab, Da, Db))
    perms = np.asarray(perms)
    assert x.shape == (N, D), x.shape
    T = _build_T(Aa, Ab, Da, Db, perms)
    W = _prep_W(T)
    x8 = _prep_x(x)
    out, _ = _run_device(x8, W, trace=False)
    return out
